# revision 4
# baseline (speedup 1.0000x reference)
"""PointGRN (segment_reduce) Trainium2 Bass kernel.

Computation (per segment b, channel c over points feat [N, 64] f32):
    sumsq[b,c]  = sum_{n in seg b} feat[n,c]^2
    r[b,c]      = sqrt(sumsq[b,c])
    rn[b,c]     = r[b,c] / (mean_c r[b,:] + 1e-6)
    out[n,c]    = feat[n,c] * (1 + gamma[c]*rn[b,c]) + beta[c]

Sharding: data-parallel over segments — host reads `offset` and gives each
of the 8 cores one whole segment (padded with zero rows to a common tile-
aligned length).  No device-side searchsorted and no collectives needed.

Device kernel (per core, R rows):
    pass 1: stream [128 x 2048] f32 tiles; ACT squares into bf16; PE
            reduces partitions via ones-matmul accumulating into PSUM.
    combine: tiny [1,64] vector math (sqrt + Newton step, mean, scale),
            broadcast scale/beta to [128,64] via a K=1 matmul.
    pass 2: re-stream tiles; DVE computes y = x*s + beta; store.
"""

import numpy as np

import concourse.bacc as bacc
import concourse.bass as bass
import concourse.mybir as mybir
import concourse.tile as tile
from concourse.bass_utils import run_bass_kernel_spmd

EPS = 1e-06
N_CORES = 8
P = 128          # SBUF partitions
C = 64           # channels
K = 32           # row-groups per partition per tile
F = K * C        # tile free dim (2048 f32 = 8KB/partition)
TILE_ROWS = P * K  # 4096 rows per tile
MM_N = 512       # matmul moving free-dim chunk
NCHUNK = F // MM_N

_AFT = mybir.ActivationFunctionType
_ALU = mybir.AluOpType

_program_cache: dict[int, bass.Bass] = {}


def _build_program(r_pad: int) -> bass.Bass:
    """One-core Bass program for a shard of r_pad rows (r_pad % TILE_ROWS == 0)."""
    from contextlib import ExitStack

    nt = r_pad // TILE_ROWS
    nc = bacc.Bacc()

    feat = nc.declare_dram_parameter("feat", [r_pad, C], mybir.dt.float32, isOutput=False)
    gamma = nc.declare_dram_parameter("gamma", [1, C], mybir.dt.float32, isOutput=False)
    beta = nc.declare_dram_parameter("beta", [1, C], mybir.dt.float32, isOutput=False)
    out = nc.declare_dram_parameter("out", [r_pad, C], mybir.dt.float32, isOutput=True)

    feat_t = feat[:].rearrange("(t p k) c -> t p (k c)", p=P, k=K)
    out_t = out[:].rearrange("(t p k) c -> t p (k c)", p=P, k=K)

    with tile.TileContext(nc) as tc, ExitStack() as ctx:
        const = ctx.enter_context(tc.tile_pool(name="const", bufs=1))
        inp = ctx.enter_context(tc.tile_pool(name="inp", bufs=4))
        sqp = ctx.enter_context(tc.tile_pool(name="sqp", bufs=3))
        outp = ctx.enter_context(tc.tile_pool(name="outp", bufs=4))
        psum = ctx.enter_context(tc.tile_pool(name="psum", bufs=1, space="PSUM"))
        small = ctx.enter_context(tc.tile_pool(name="small", bufs=1))

        ones_col = const.tile([P, 1], mybir.dt.bfloat16, name="ones_col", tag="ones_col")
        nc.vector.memset(ones_col, 1.0)
        ones_row = const.tile([1, P], mybir.dt.float32, name="ones_row", tag="ones_row")
        nc.vector.memset(ones_row, 1.0)

        # PSUM accumulators: acc[j][0, i] accumulates sum over tiles/partitions
        # of squared elements at within-chunk free position i (= k_local*64+c).
        acc = [
            psum.tile([1, MM_N], mybir.dt.float32, name=f"acc{j}", tag=f"acc{j}")
            for j in range(NCHUNK)
        ]

        # ---- pass 1: sum of squares --------------------------------------
        for t in range(nt):
            x = inp.tile([P, F], mybir.dt.float32, name="x", tag="x")
            nc.sync.dma_start(out=x, in_=feat_t[t])
            sq = sqp.tile([P, F], mybir.dt.bfloat16, name="sq", tag="sq")
            nc.scalar.activation(sq, x, _AFT.Square)
            for j in range(NCHUNK):
                nc.tensor.matmul(
                    acc[j][:, :],
                    lhsT=ones_col[:, :],
                    rhs=sq[:, j * MM_N : (j + 1) * MM_N],
                    start=(t == 0),
                    stop=(t == nt - 1),
                )

        # ---- combine: [1,64] vector math ---------------------------------
        # per-chunk reduce over k_local (stride 64, innermost) -> [1, 64]
        red = small.tile([1, NCHUNK, C], mybir.dt.float32, name="red", tag="red")
        for j in range(NCHUNK):
            nc.vector.tensor_reduce(
                out=red[:, j, :],
                in_=acc[j][:, :].rearrange("p (k c) -> p c k", c=C),
                axis=mybir.AxisListType.X,
                op=_ALU.add,
            )
        sumsq = small.tile([1, C], mybir.dt.float32, name="sumsq", tag="sumsq")
        nc.vector.tensor_reduce(
            out=sumsq,
            in_=red[:, :, :].rearrange("p k c -> p c k"),
            axis=mybir.AxisListType.X,
            op=_ALU.add,
        )

        # r2 = 2*sqrt(sumsq) via ACT sqrt + one Newton step (ACT sqrt is low
        # precision; Newton with the accurate DVE reciprocal fixes it):
        #   r0 = sqrt_act(d); r2 = r0 + d / max(r0, tiny)
        r0 = small.tile([1, C], mybir.dt.float32, name="r0", tag="r0")
        nc.scalar.activation(r0, sumsq, _AFT.Sqrt)
        rm = small.tile([1, C], mybir.dt.float32, name="rm", tag="rm")
        nc.vector.tensor_scalar_max(rm, r0, 1e-30)
        rinv = small.tile([1, C], mybir.dt.float32, name="rinv", tag="rinv")
        nc.vector.reciprocal(rinv, rm)
        t1 = small.tile([1, C], mybir.dt.float32, name="t1", tag="t1")
        nc.vector.tensor_mul(t1, sumsq, rinv)
        r2 = small.tile([1, C], mybir.dt.float32, name="r2", tag="r2")
        nc.vector.tensor_add(r2, r0, t1)

        # mean + eps:  me = sum(r2)/128 + EPS   (r2 = 2r, mean = sum(r2)/128)
        msum = small.tile([1, 1], mybir.dt.float32, name="msum", tag="msum")
        nc.vector.tensor_reduce(out=msum, in_=r2, axis=mybir.AxisListType.X, op=_ALU.add)
        eps_t = small.tile([1, 1], mybir.dt.float32, name="eps_t", tag="eps_t")
        nc.vector.memset(eps_t, EPS)
        me = small.tile([1, 1], mybir.dt.float32, name="me", tag="me")
        nc.scalar.activation(me, msum, _AFT.Identity, bias=eps_t[:, :], scale=1.0 / (2 * C))
        minv = small.tile([1, 1], mybir.dt.float32, name="minv", tag="minv")
        nc.vector.reciprocal(minv, me)
        mh = small.tile([1, 1], mybir.dt.float32, name="mh", tag="mh")
        nc.vector.tensor_scalar_mul(mh, minv, 0.5)

        # s = 1 + gamma * (r2 * 0.5 * minv)
        g_row = small.tile([1, C], mybir.dt.float32, name="g_row", tag="g_row")
        nc.sync.dma_start(out=g_row, in_=gamma[:])
        t2 = small.tile([1, C], mybir.dt.float32, name="t2", tag="t2")
        nc.vector.tensor_mul(t2, r2, g_row)
        sb_cat = small.tile([1, 2 * C], mybir.dt.float32, name="sb_cat", tag="sb_cat")
        nc.vector.tensor_scalar(
            sb_cat[:, 0:C], t2, scalar1=mh[:, :], scalar2=1.0, op0=_ALU.mult, op1=_ALU.add
        )
        nc.sync.dma_start(out=sb_cat[:, C : 2 * C], in_=beta[:])

        # broadcast [1,128] -> [128,128] with a K=1 matmul: cols 0-63 = s, 64-127 = beta
        bc_ps = psum.tile([P, 2 * C], mybir.dt.float32, name="bc_ps", tag="bc_ps")
        nc.tensor.matmul(bc_ps[:, :], lhsT=ones_row[:, :], rhs=sb_cat[:, :], start=True, stop=True)
        sb_bc = const.tile([P, 2 * C], mybir.dt.float32, name="sb_bc", tag="sb_bc")
        nc.scalar.copy(sb_bc, bc_ps)
        s_bc = sb_bc[:, 0:C]
        b_bc = sb_bc[:, C : 2 * C]
        # broadcast views over the K row-groups of a tile's free axis
        s_ap = bass.AP(tensor=s_bc.tensor, offset=s_bc.offset, ap=[s_bc.ap[0], [0, K], s_bc.ap[1]])
        b_ap = bass.AP(tensor=b_bc.tensor, offset=b_bc.offset, ap=[b_bc.ap[0], [0, K], b_bc.ap[1]])

        # ---- pass 2: y = x*s + beta --------------------------------------
        for t in range(nt):
            x = inp.tile([P, F], mybir.dt.float32, name="x", tag="x")
            nc.sync.dma_start(out=x, in_=feat_t[t])
            y = outp.tile([P, F], mybir.dt.float32, name="y", tag="y")
            nc.vector.tensor_tensor(
                y[:, :].rearrange("p (k c) -> p k c", c=C),
                x[:, :].rearrange("p (k c) -> p k c", c=C),
                s_ap,
                _ALU.mult,
            )
            nc.vector.tensor_tensor(
                y[:, :].rearrange("p (k c) -> p k c", c=C),
                y[:, :].rearrange("p (k c) -> p k c", c=C),
                b_ap,
                _ALU.add,
            )
            nc.sync.dma_start(out=out_t[t], in_=y)

    nc.finalize()
    return nc


def kernel(feat: np.ndarray, offset: np.ndarray, gamma: np.ndarray, beta: np.ndarray) -> np.ndarray:
    feat = np.ascontiguousarray(np.asarray(feat, dtype=np.float32))
    offset = np.asarray(offset)
    gamma = np.ascontiguousarray(np.asarray(gamma, dtype=np.float32)).reshape(1, C)
    beta = np.ascontiguousarray(np.asarray(beta, dtype=np.float32)).reshape(1, C)

    n = feat.shape[0]
    b = offset.shape[0]
    assert b <= N_CORES, f"need <= {N_CORES} segments, got {b}"

    ends = offset.astype(np.int64)
    starts = np.concatenate([[0], ends[:-1]])
    seg_rows = (ends - starts).astype(np.int64)

    r_max = int(seg_rows.max()) if b else TILE_ROWS
    r_pad = max(TILE_ROWS, ((r_max + TILE_ROWS - 1) // TILE_ROWS) * TILE_ROWS)

    nc = _program_cache.get(r_pad)
    if nc is None:
        nc = _build_program(r_pad)
        _program_cache[r_pad] = nc

    in_maps = []
    for i in range(N_CORES):
        shard = np.zeros((r_pad, C), dtype=np.float32)
        if i < b and seg_rows[i] > 0:
            shard[: seg_rows[i]] = feat[starts[i] : ends[i]]
        in_maps.append({"feat": shard, "gamma": gamma, "beta": beta})

    results = run_bass_kernel_spmd(nc, in_maps, core_ids=list(range(N_CORES))).results

    out_full = np.empty((n, C), dtype=np.float32)
    for i in range(b):
        if seg_rows[i] > 0:
            out_full[starts[i] : ends[i]] = results[i]["out"][: seg_rows[i]]
    return out_full


# revision 7
# speedup vs baseline: 339.2939x; 339.2939x over previous
"""PointGRN (segment_reduce) Trainium2 Bass kernel.

Computation (per segment b, channel c over points feat [N, 64] f32):
    sumsq[b,c]  = sum_{n in seg b} feat[n,c]^2
    r[b,c]      = sqrt(sumsq[b,c])
    rn[b,c]     = r[b,c] / (mean_c r[b,:] + 1e-6)
    out[n,c]    = feat[n,c] * (1 + gamma[c]*rn[b,c]) + beta[c]

Sharding: data-parallel over segments — host reads `offset` and gives each
of the 8 cores one whole segment (padded with zero rows to a common tile-
aligned length).  No device-side searchsorted and no collectives needed.

Device kernel (per core, R rows):
    pass 1: stream [128 x 2048] f32 tiles; ACT squares into bf16; PE
            reduces partitions via ones-matmul accumulating into PSUM.
    combine: tiny [1,64] vector math (sqrt + Newton step, mean, scale),
            broadcast scale/beta to [128,64] via a K=1 matmul.
    pass 2: re-stream tiles; DVE computes y = x*s + beta; store.
"""

import numpy as np

import concourse.bacc as bacc
import concourse.bass as bass
import concourse.mybir as mybir
import concourse.tile as tile
from concourse.bass_utils import run_bass_kernel_spmd

EPS = 1e-06
N_CORES = 8
P = 128          # SBUF partitions
C = 64           # channels
K = 32           # row-groups per partition per tile
F = K * C        # tile free dim (2048 f32 = 8KB/partition)
TILE_ROWS = P * K  # 4096 rows per tile
MM_N = 512       # matmul moving free-dim chunk
NCHUNK = F // MM_N

_AFT = mybir.ActivationFunctionType
_ALU = mybir.AluOpType

_program_cache: dict[int, bass.Bass] = {}


def _build_program(r_pad: int, repeats: int = 1) -> bass.Bass:
    """One-core Bass program for a shard of r_pad rows (r_pad % TILE_ROWS == 0).

    `repeats` re-runs the whole computation body that many times (timing only:
    the wall-clock slope over repeats isolates kernel time from dispatch
    overhead, which is ~97ms flat in this axon environment).
    """
    from contextlib import ExitStack

    nt = r_pad // TILE_ROWS
    nc = bacc.Bacc()

    feat = nc.declare_dram_parameter("feat", [r_pad, C], mybir.dt.float32, isOutput=False)
    gamma = nc.declare_dram_parameter("gamma", [1, C], mybir.dt.float32, isOutput=False)
    beta = nc.declare_dram_parameter("beta", [1, C], mybir.dt.float32, isOutput=False)
    out = nc.declare_dram_parameter("out", [r_pad, C], mybir.dt.float32, isOutput=True)

    feat_t = feat[:].rearrange("(t p k) c -> t p (k c)", p=P, k=K)
    out_t = out[:].rearrange("(t p k) c -> t p (k c)", p=P, k=K)

    with tile.TileContext(nc) as tc, ExitStack() as ctx:
        const = ctx.enter_context(tc.tile_pool(name="const", bufs=1))
        inp = ctx.enter_context(tc.tile_pool(name="inp", bufs=4))
        sqp = ctx.enter_context(tc.tile_pool(name="sqp", bufs=3))
        outp = ctx.enter_context(tc.tile_pool(name="outp", bufs=4))
        psum = ctx.enter_context(tc.tile_pool(name="psum", bufs=1, space="PSUM"))
        small = ctx.enter_context(tc.tile_pool(name="small", bufs=1))

        ones_col = const.tile([P, 1], mybir.dt.bfloat16, name="ones_col", tag="ones_col")
        nc.vector.memset(ones_col, 1.0)
        ones_row = const.tile([1, P], mybir.dt.float32, name="ones_row", tag="ones_row")
        nc.vector.memset(ones_row, 1.0)

        for _rep in range(repeats):
            _build_body(nc, tc, feat, gamma, beta, feat_t, out_t, nt,
                        const, inp, sqp, outp, psum, small, ones_col, ones_row)

    nc.finalize()
    return nc


def _build_body(nc, tc, feat, gamma, beta, feat_t, out_t, nt,
                const, inp, sqp, outp, psum, small, ones_col, ones_row):
    if True:  # keep original indentation
        # PSUM accumulators: acc[j][0, i] accumulates sum over tiles/partitions
        # of squared elements at within-chunk free position i (= k_local*64+c).
        acc = [
            psum.tile([1, MM_N], mybir.dt.float32, name=f"acc{j}", tag=f"acc{j}")
            for j in range(NCHUNK)
        ]

        # ---- pass 1: sum of squares --------------------------------------
        for t in range(nt):
            x = inp.tile([P, F], mybir.dt.float32, name="x", tag="x")
            nc.sync.dma_start(out=x, in_=feat_t[t])
            sq = sqp.tile([P, F], mybir.dt.bfloat16, name="sq", tag="sq")
            nc.scalar.activation(sq, x, _AFT.Square)
            for j in range(NCHUNK):
                nc.tensor.matmul(
                    acc[j][:, :],
                    lhsT=ones_col[:, :],
                    rhs=sq[:, j * MM_N : (j + 1) * MM_N],
                    start=(t == 0),
                    stop=(t == nt - 1),
                )

        # ---- combine: [1,64] vector math ---------------------------------
        # per-chunk reduce over k_local (stride 64, innermost) -> [1, 64]
        red = small.tile([1, NCHUNK, C], mybir.dt.float32, name="red", tag="red")
        for j in range(NCHUNK):
            nc.vector.tensor_reduce(
                out=red[:, j, :],
                in_=acc[j][:, :].rearrange("p (k c) -> p c k", c=C),
                axis=mybir.AxisListType.X,
                op=_ALU.add,
            )
        sumsq = small.tile([1, C], mybir.dt.float32, name="sumsq", tag="sumsq")
        nc.vector.tensor_reduce(
            out=sumsq,
            in_=red[:, :, :].rearrange("p k c -> p c k"),
            axis=mybir.AxisListType.X,
            op=_ALU.add,
        )

        # r2 = 2*sqrt(sumsq) via ACT sqrt + one Newton step (ACT sqrt is low
        # precision; Newton with the accurate DVE reciprocal fixes it):
        #   r0 = sqrt_act(d); r2 = r0 + d / max(r0, tiny)
        r0 = small.tile([1, C], mybir.dt.float32, name="r0", tag="r0")
        nc.scalar.activation(r0, sumsq, _AFT.Sqrt)
        rm = small.tile([1, C], mybir.dt.float32, name="rm", tag="rm")
        nc.vector.tensor_scalar_max(rm, r0, 1e-30)
        rinv = small.tile([1, C], mybir.dt.float32, name="rinv", tag="rinv")
        nc.vector.reciprocal(rinv, rm)
        t1 = small.tile([1, C], mybir.dt.float32, name="t1", tag="t1")
        nc.vector.tensor_mul(t1, sumsq, rinv)
        r2 = small.tile([1, C], mybir.dt.float32, name="r2", tag="r2")
        nc.vector.tensor_add(r2, r0, t1)

        # mean + eps:  me = sum(r2)/128 + EPS   (r2 = 2r, mean = sum(r2)/128)
        msum = small.tile([1, 1], mybir.dt.float32, name="msum", tag="msum")
        nc.vector.tensor_reduce(out=msum, in_=r2, axis=mybir.AxisListType.X, op=_ALU.add)
        eps_t = small.tile([1, 1], mybir.dt.float32, name="eps_t", tag="eps_t")
        nc.vector.memset(eps_t, EPS)
        me = small.tile([1, 1], mybir.dt.float32, name="me", tag="me")
        nc.scalar.activation(me, msum, _AFT.Identity, bias=eps_t[:, :], scale=1.0 / (2 * C))
        minv = small.tile([1, 1], mybir.dt.float32, name="minv", tag="minv")
        nc.vector.reciprocal(minv, me)
        mh = small.tile([1, 1], mybir.dt.float32, name="mh", tag="mh")
        nc.vector.tensor_scalar_mul(mh, minv, 0.5)

        # s = 1 + gamma * (r2 * 0.5 * minv)
        g_row = small.tile([1, C], mybir.dt.float32, name="g_row", tag="g_row")
        nc.sync.dma_start(out=g_row, in_=gamma[:])
        t2 = small.tile([1, C], mybir.dt.float32, name="t2", tag="t2")
        nc.vector.tensor_mul(t2, r2, g_row)
        sb_cat = small.tile([1, 2 * C], mybir.dt.float32, name="sb_cat", tag="sb_cat")
        nc.vector.tensor_scalar(
            sb_cat[:, 0:C], t2, scalar1=mh[:, :], scalar2=1.0, op0=_ALU.mult, op1=_ALU.add
        )
        nc.sync.dma_start(out=sb_cat[:, C : 2 * C], in_=beta[:])

        # broadcast [1,128] -> [128,128] with a K=1 matmul: cols 0-63 = s, 64-127 = beta
        bc_ps = psum.tile([P, 2 * C], mybir.dt.float32, name="bc_ps", tag="bc_ps")
        nc.tensor.matmul(bc_ps[:, :], lhsT=ones_row[:, :], rhs=sb_cat[:, :], start=True, stop=True)
        sb_bc = const.tile([P, 2 * C], mybir.dt.float32, name="sb_bc", tag="sb_bc")
        nc.scalar.copy(sb_bc, bc_ps)
        s_bc = sb_bc[:, 0:C]
        b_bc = sb_bc[:, C : 2 * C]
        # broadcast views over the K row-groups of a tile's free axis
        s_ap = bass.AP(tensor=s_bc.tensor, offset=s_bc.offset, ap=[s_bc.ap[0], [0, K], s_bc.ap[1]])
        b_ap = bass.AP(tensor=b_bc.tensor, offset=b_bc.offset, ap=[b_bc.ap[0], [0, K], b_bc.ap[1]])

        # ---- pass 2: y = x*s + beta --------------------------------------
        for t in range(nt):
            x = inp.tile([P, F], mybir.dt.float32, name="x", tag="x")
            nc.sync.dma_start(out=x, in_=feat_t[t])
            y = outp.tile([P, F], mybir.dt.float32, name="y", tag="y")
            nc.vector.tensor_tensor(
                y[:, :].rearrange("p (k c) -> p k c", c=C),
                x[:, :].rearrange("p (k c) -> p k c", c=C),
                s_ap,
                _ALU.mult,
            )
            nc.vector.tensor_tensor(
                y[:, :].rearrange("p (k c) -> p k c", c=C),
                y[:, :].rearrange("p (k c) -> p k c", c=C),
                b_ap,
                _ALU.add,
            )
            nc.sync.dma_start(out=out_t[t], in_=y)


def kernel(feat: np.ndarray, offset: np.ndarray, gamma: np.ndarray, beta: np.ndarray) -> np.ndarray:
    feat = np.ascontiguousarray(np.asarray(feat, dtype=np.float32))
    offset = np.asarray(offset)
    gamma = np.ascontiguousarray(np.asarray(gamma, dtype=np.float32)).reshape(1, C)
    beta = np.ascontiguousarray(np.asarray(beta, dtype=np.float32)).reshape(1, C)

    n = feat.shape[0]
    b = offset.shape[0]
    assert b <= N_CORES, f"need <= {N_CORES} segments, got {b}"

    ends = offset.astype(np.int64)
    starts = np.concatenate([[0], ends[:-1]])
    seg_rows = (ends - starts).astype(np.int64)

    r_max = int(seg_rows.max()) if b else TILE_ROWS
    r_pad = max(TILE_ROWS, ((r_max + TILE_ROWS - 1) // TILE_ROWS) * TILE_ROWS)

    nc = _program_cache.get(r_pad)
    if nc is None:
        nc = _build_program(r_pad)
        _program_cache[r_pad] = nc

    in_maps = []
    for i in range(N_CORES):
        shard = np.zeros((r_pad, C), dtype=np.float32)
        if i < b and seg_rows[i] > 0:
            shard[: seg_rows[i]] = feat[starts[i] : ends[i]]
        in_maps.append({"feat": shard, "gamma": gamma, "beta": beta})

    results = run_bass_kernel_spmd(nc, in_maps, core_ids=list(range(N_CORES))).results

    out_full = np.empty((n, C), dtype=np.float32)
    for i in range(b):
        if seg_rows[i] > 0:
            out_full[starts[i] : ends[i]] = results[i]["out"][: seg_rows[i]]
    return out_full


# revision 19
# speedup vs baseline: 365.2222x; 1.0764x over previous
"""PointGRN (segment_reduce) Trainium2 Bass kernel.

Computation (per segment b, channel c over points feat [N, 64] f32):
    sumsq[b,c]  = sum_{n in seg b} feat[n,c]^2
    r[b,c]      = sqrt(sumsq[b,c])
    rn[b,c]     = r[b,c] / (mean_c r[b,:] + 1e-6)
    out[n,c]    = feat[n,c] * (1 + gamma[c]*rn[b,c]) + beta[c]

Sharding: data-parallel over segments — host reads `offset` and gives each
of the 8 cores one whole segment (padded with zero rows to a common tile-
aligned length).  No device-side searchsorted and no collectives needed.

Device kernel (per core, R rows), DMA-bound at ~330-400 GB/s/core:
    pass 1: stream [128 x K*64] f32 tiles; ACT squares into bf16; PE ones-
            matmul reduces partitions, accumulating into 4 PSUM rows.
            The first RES tiles stay resident in SBUF.
    combine: tiny [1,64] vector math (sqrt + Newton step, mean, scale),
            broadcast scale/beta to [128,64] via a K=1 matmul.
    pass 2: resident tiles are rescaled in place (no reload); the rest are
            re-streamed; y = x*s + beta; store.  Loads ride the SP HWDGE
            ring, stores the ACT ring, halving per-ring DMA count.
"""

import numpy as np

import concourse.bacc as bacc
import concourse.bass as bass
import concourse.mybir as mybir
import concourse.tile as tile
from concourse.bass_utils import run_bass_kernel_spmd

EPS = 1e-06
N_CORES = 8
P = 128          # SBUF partitions
C = 64           # channels
K = 32           # row-groups per partition per tile
F = K * C        # tile free dim (2048 f32 = 8KB/partition)
TILE_ROWS = P * K  # 4096 rows per tile
MM_N = 512       # matmul moving free-dim chunk
NCHUNK = F // MM_N
RES = 19         # tiles kept resident in SBUF between the two passes

_AFT = mybir.ActivationFunctionType
_ALU = mybir.AluOpType

_program_cache: dict[tuple, bass.Bass] = {}


def _build_program(
    r_pad: int,
    repeats: int = 1,
    res: int = RES,
    add_eng: str = "gpsimd",
    bufs_x: int = 3,
    alternate_rings: bool = True,
) -> bass.Bass:
    """One-core Bass program for a shard of r_pad rows (r_pad % TILE_ROWS == 0).

    `repeats` re-runs the whole computation body that many times (timing only:
    the wall-clock slope over repeats isolates kernel time from dispatch
    overhead, which is ~80-100ms flat in this axon environment).
    """
    from contextlib import ExitStack

    nt = r_pad // TILE_ROWS
    res = min(res, nt)
    nc = bacc.Bacc()

    feat = nc.declare_dram_parameter("feat", [r_pad, C], mybir.dt.float32, isOutput=False)
    gamma = nc.declare_dram_parameter("gamma", [1, C], mybir.dt.float32, isOutput=False)
    beta = nc.declare_dram_parameter("beta", [1, C], mybir.dt.float32, isOutput=False)
    out = nc.declare_dram_parameter("out", [r_pad, C], mybir.dt.float32, isOutput=True)

    feat_t = feat[:].rearrange("(t p k) c -> t p (k c)", p=P, k=K)
    out_t = out[:].rearrange("(t p k) c -> t p (k c)", p=P, k=K)

    # Alternate big DMAs across the two HWDGE rings (SP + ACT): a single
    # ring measures ~305 GB/s, both together ~332 GB/s on this part.
    _ring_state = [0]

    def dma(out_ap, in_ap):
        if alternate_rings:
            eng = nc.sync if _ring_state[0] % 2 == 0 else nc.scalar
            _ring_state[0] += 1
        else:
            eng = nc.sync
        eng.dma_start(out=out_ap, in_=in_ap)

    with tile.TileContext(nc) as tc, ExitStack() as ctx:
        const = ctx.enter_context(tc.tile_pool(name="const", bufs=1))
        inp = ctx.enter_context(tc.tile_pool(name="inp", bufs=bufs_x))
        resp = ctx.enter_context(tc.tile_pool(name="resp", bufs=1))
        sqp = ctx.enter_context(tc.tile_pool(name="sqp", bufs=2))
        psum = ctx.enter_context(tc.tile_pool(name="psum", bufs=1, space="PSUM"))
        small = ctx.enter_context(tc.tile_pool(name="small", bufs=1))
        adder = getattr(nc, add_eng)

        ones_col = const.tile([P, 1], mybir.dt.bfloat16, name="ones_col", tag="ones_col")
        nc.vector.memset(ones_col, 1.0)
        ones_row = const.tile([1, P], mybir.dt.float32, name="ones_row", tag="ones_row")
        nc.vector.memset(ones_row, 1.0)

        for _rep in range(repeats):
            # --- pass 1: sum of squares ----------------------------------
            acc = [
                psum.tile([1, MM_N], mybir.dt.float32, name=f"acc{j}", tag=f"acc{j}")
                for j in range(NCHUNK)
            ]
            res_tiles = []
            for t in range(nt):
                if t < res:
                    x = resp.tile([P, F], mybir.dt.float32, name="xr", tag=f"res{t}")
                    res_tiles.append(x)
                else:
                    x = inp.tile([P, F], mybir.dt.float32, name="x", tag="x")
                dma(x, feat_t[t])
                sq = sqp.tile([P, F], mybir.dt.bfloat16, name="sq", tag="sq")
                nc.scalar.activation(sq, x, _AFT.Square)
                for j in range(NCHUNK):
                    nc.tensor.matmul(
                        acc[j][:, :],
                        lhsT=ones_col[:, :],
                        rhs=sq[:, j * MM_N : (j + 1) * MM_N],
                        start=(t == 0),
                        stop=(t == nt - 1),
                    )

            # --- combine: [1,64] vector math ------------------------------
            red = small.tile([1, NCHUNK, C], mybir.dt.float32, name="red", tag="red")
            for j in range(NCHUNK):
                nc.vector.tensor_reduce(
                    out=red[:, j, :],
                    in_=acc[j][:, :].rearrange("p (k c) -> p c k", c=C),
                    axis=mybir.AxisListType.X,
                    op=_ALU.add,
                )
            sumsq = small.tile([1, C], mybir.dt.float32, name="sumsq", tag="sumsq")
            nc.vector.tensor_reduce(
                out=sumsq,
                in_=red[:, :, :].rearrange("p k c -> p c k"),
                axis=mybir.AxisListType.X,
                op=_ALU.add,
            )

            # r2 = 2*sqrt(sumsq) via ACT sqrt + one Newton step (ACT sqrt is
            # low precision; Newton with the accurate DVE reciprocal fixes it)
            r0 = small.tile([1, C], mybir.dt.float32, name="r0", tag="r0")
            nc.scalar.activation(r0, sumsq, _AFT.Sqrt)
            rm = small.tile([1, C], mybir.dt.float32, name="rm", tag="rm")
            nc.vector.tensor_scalar_max(rm, r0, 1e-30)
            rinv = small.tile([1, C], mybir.dt.float32, name="rinv", tag="rinv")
            nc.vector.reciprocal(rinv, rm)
            t1 = small.tile([1, C], mybir.dt.float32, name="t1", tag="t1")
            nc.vector.tensor_mul(t1, sumsq, rinv)
            r2 = small.tile([1, C], mybir.dt.float32, name="r2", tag="r2")
            nc.vector.tensor_add(r2, r0, t1)

            # mean + eps:  me = sum(r2)/128 + EPS   (r2 = 2r -> mean = sum/128)
            msum = small.tile([1, 1], mybir.dt.float32, name="msum", tag="msum")
            nc.vector.tensor_reduce(out=msum, in_=r2, axis=mybir.AxisListType.X, op=_ALU.add)
            eps_t = small.tile([1, 1], mybir.dt.float32, name="eps_t", tag="eps_t")
            nc.vector.memset(eps_t, EPS)
            me = small.tile([1, 1], mybir.dt.float32, name="me", tag="me")
            nc.scalar.activation(me, msum, _AFT.Identity, bias=eps_t[:, :], scale=1.0 / (2 * C))
            minv = small.tile([1, 1], mybir.dt.float32, name="minv", tag="minv")
            nc.vector.reciprocal(minv, me)
            mh = small.tile([1, 1], mybir.dt.float32, name="mh", tag="mh")
            nc.vector.tensor_scalar_mul(mh, minv, 0.5)

            # s = 1 + gamma * (r2 * 0.5 * minv); pack [s | beta] in one row
            g_row = small.tile([1, C], mybir.dt.float32, name="g_row", tag="g_row")
            nc.sync.dma_start(out=g_row, in_=gamma[:])
            t2 = small.tile([1, C], mybir.dt.float32, name="t2", tag="t2")
            nc.vector.tensor_mul(t2, r2, g_row)
            sb_cat = small.tile([1, 2 * C], mybir.dt.float32, name="sb_cat", tag="sb_cat")
            nc.vector.tensor_scalar(
                sb_cat[:, 0:C], t2, scalar1=mh[:, :], scalar2=1.0, op0=_ALU.mult, op1=_ALU.add
            )
            nc.sync.dma_start(out=sb_cat[:, C : 2 * C], in_=beta[:])

            # broadcast [1,128] -> [128,128]: cols 0-63 = s, 64-127 = beta
            bc_ps = psum.tile([P, 2 * C], mybir.dt.float32, name="bc_ps", tag="bc_ps")
            nc.tensor.matmul(bc_ps[:, :], lhsT=ones_row[:, :], rhs=sb_cat[:, :], start=True, stop=True)
            sb_bc = small.tile([P, 2 * C], mybir.dt.float32, name="sb_bc", tag="sb_bc")
            nc.scalar.copy(sb_bc, bc_ps)
            s_bc = sb_bc[:, 0:C]
            b_bc = sb_bc[:, C : 2 * C]
            # broadcast views over the K row-groups of a tile's free axis
            s_ap = bass.AP(tensor=s_bc.tensor, offset=s_bc.offset, ap=[s_bc.ap[0], [0, K], s_bc.ap[1]])
            b_ap = bass.AP(tensor=b_bc.tensor, offset=b_bc.offset, ap=[b_bc.ap[0], [0, K], b_bc.ap[1]])

            def rescale(x, t):
                x3 = x[:, :].rearrange("p (k c) -> p k c", c=C)
                nc.vector.tensor_tensor(x3, x3, s_ap, _ALU.mult)
                # alternate the +beta between Pool and DVE so neither engine
                # becomes the pass-2 critical path
                eng = adder if t % 2 == 0 else nc.vector
                eng.tensor_tensor(x3, x3, b_ap, _ALU.add)

            # --- pass 2: y = x*s + beta (in place) ------------------------
            # resident tiles first (no loads; fills the combine bubble while
            # streamed loads prefetch), stored from SBUF directly
            for t in range(res):
                x = res_tiles[t]
                rescale(x, t)
                dma(out_t[t], x)
            for t in range(res, nt):
                x = inp.tile([P, F], mybir.dt.float32, name="x", tag="x")
                dma(x, feat_t[t])
                rescale(x, t)
                dma(out_t[t], x)

    nc.finalize()
    return nc


def kernel(feat: np.ndarray, offset: np.ndarray, gamma: np.ndarray, beta: np.ndarray) -> np.ndarray:
    feat = np.ascontiguousarray(np.asarray(feat, dtype=np.float32))
    offset = np.asarray(offset)
    gamma = np.ascontiguousarray(np.asarray(gamma, dtype=np.float32)).reshape(1, C)
    beta = np.ascontiguousarray(np.asarray(beta, dtype=np.float32)).reshape(1, C)

    n = feat.shape[0]
    b = offset.shape[0]
    assert b <= N_CORES, f"need <= {N_CORES} segments, got {b}"

    ends = offset.astype(np.int64)
    starts = np.concatenate([[0], ends[:-1]])
    seg_rows = (ends - starts).astype(np.int64)

    r_max = int(seg_rows.max()) if b else TILE_ROWS
    r_pad = max(TILE_ROWS, ((r_max + TILE_ROWS - 1) // TILE_ROWS) * TILE_ROWS)

    key = (r_pad,)
    nc = _program_cache.get(key)
    if nc is None:
        nc = _build_program(r_pad)
        _program_cache[key] = nc

    in_maps = []
    for i in range(N_CORES):
        shard = np.zeros((r_pad, C), dtype=np.float32)
        if i < b and seg_rows[i] > 0:
            shard[: seg_rows[i]] = feat[starts[i] : ends[i]]
        in_maps.append({"feat": shard, "gamma": gamma, "beta": beta})

    results = run_bass_kernel_spmd(nc, in_maps, core_ids=list(range(N_CORES))).results

    out_full = np.empty((n, C), dtype=np.float32)
    for i in range(b):
        if seg_rows[i] > 0:
            out_full[starts[i] : ends[i]] = results[i]["out"][: seg_rows[i]]
    return out_full
